# revision 1
# baseline (speedup 1.0000x reference)
"""Trainium2 Bass kernel for nn_BinaryFinCast (patch-embed + 12-layer MoE
transformer + binary head), data-parallel over batch across 8 NeuronCores.

Contract: kernel(**inputs) takes the FULL unsharded inputs (numpy arrays,
keyed as in setup_inputs()) and returns the FULL output
(logits[16] fp32, sigmoid(logits)[16] fp32).

Design notes:
  - Pure data parallelism: 16 sequences / 8 cores = 2 per core; each core
    runs the whole network on its 2 sequences.  No collectives.
  - Activations are feature-major ([D partitions, tokens free]) so matmuls
    chain without transposes.  LayerNorm stats (partition-dim reductions)
    use ones-vector matmuls; per-token rows broadcast back with K=1 matmuls.
  - Matmul inputs bf16 (fp32 PSUM accumulation); residual stream and all
    normalization statistics stay fp32.
  - Attention computes transposed scores sT[k,q] = kT.T @ qT so the softmax
    reduction runs over the partition dim via ones-matmuls; the causal mask
    is a multiplicative upper-triangular constant; per-head o comes out
    feature-major directly (lhsT = token-major v).
  - MoE: dense evaluation of all 4 experts; top-2 combine weights are
    computed on-device token-major, transposed, broadcast via one-hot
    matmuls, and folded into the w2 matmul inputs, so each expert's
    contribution (plus the combined b2 bias) accumulates in PSUM.
"""

import numpy as np
import ml_dtypes

# ---------------------------------------------------------------- shapes
B, S, C = 16, 2048, 8
P, D, NH, L, E, TOPK, H = 16, 512, 8, 12, 4, 2, 2048
PD = P * C            # 128
IRH = 512
N = S // P            # 128 tokens per sequence
NCORES = 8
BPC = B // NCORES     # 2 sequences per core
TOK = BPC * N         # 256 token columns per core
DH = D // NH          # 64
KT = D // 128         # 4
HT = H // 128         # 16

F32 = np.float32
BF16 = np.float16

_CACHE = {}
SIM_ACT_SWAP = False  # debug: replace Gelu with Tanh (CoreSim lacks Gelu)


# ----------------------------------------------------- tile tail-drain fix
def _fixed_tile_context():
    """Stock TileContext._drain_and_barrier attaches every outstanding
    global-clock wait to a single InstDrain; this walrus build encodes only
    ~2 sync waits per instruction ("Too many sync wait commands").  Split
    the waits across single-wait carrier drains."""
    import bass_rust as _br
    import concourse.tile as tile
    from concourse.vector_clock import ScopedClock

    class FixedTileContext(tile.TileContext):
        def _drain_and_barrier(self, tick_clock, wait_clock):
            nc = self.nc
            carrier = nc.sync.drain()
            wait_clock.add_sem_waits(
                carrier.ins, ScopedClock({None: tick_clock.global_clock})
            )
            si = carrier.ins.sync_info
            waits = list(si.on_wait) if si is not None and si.on_wait else []
            if len(waits) > 1:
                carrier.ins.sync_info = _br.SyncInfo(
                    on_wait=waits[:1],
                    on_update=list(si.on_update) if si.on_update else [],
                )
                for w in waits[1:]:
                    extra = nc.sync.drain()
                    extra.ins.sync_info = _br.SyncInfo(on_wait=[w], on_update=[])
            nc.all_engine_barrier()
            assert self.sems is not None
            popped = nc._tile_sem_poison_stack.pop()
            assert popped is self._sem_poison
            nc.clear_and_free_semaphores(list(self.sems.allocated().values()))
            nc.all_engine_barrier()

    return FixedTileContext


# ------------------------------------------------------------- host packing
def _pack(w):
    """[K, M] weight -> [128, (K//128)*M]; K-tile kt at cols [kt*M,(kt+1)*M)."""
    K, M = w.shape
    kt = K // 128
    return np.ascontiguousarray(
        w.reshape(kt, 128, M).transpose(1, 0, 2).reshape(128, kt * M)
    )


def _col(v):
    """[Dim] per-feature vector -> [128, Dim//128] column layout."""
    return np.ascontiguousarray(np.asarray(v, F32).reshape(-1, 128).T)


class _Packer:
    def __init__(self, rows, dtype):
        self.rows, self.dtype = rows, dtype
        self.blocks, self.off, self.cols = [], {}, 0

    def add(self, name, arr):
        assert arr.ndim == 2 and arr.shape[0] <= self.rows, (name, arr.shape)
        self.off[name] = self.cols
        self.cols += arr.shape[1]
        self.blocks.append(np.asarray(arr))

    def finish(self):
        out = np.zeros((self.rows, self.cols), dtype=self.dtype)
        c = 0
        for a in self.blocks:
            out[: a.shape[0], c : c + a.shape[1]] = a
            c += a.shape[1]
        return out


def _prep_host(inp):
    f = lambda k: np.asarray(inp[k], F32)

    wts = _Packer(128, BF16)      # bf16 matmul weights
    bia = _Packer(128, F32)       # fp32 per-feature columns (incl. head_w)
    rows = _Packer(1, BF16)       # bf16 row-layout biases

    wts.add("ir_w1", _pack(f("ir_w1")))
    wts.add("ir_w2", _pack(f("ir_w2")))
    wts.add("p2m_w", _pack(f("p2m_w")))
    qkv_w, out_w, gate_w = f("qkv_w"), f("out_w"), f("gate_w")
    e_w1, e_w2 = f("exp_w1"), f("exp_w2")
    for l in range(L):
        wts.add(f"wq{l}", _pack(qkv_w[l][:, 0:D]))
        wts.add(f"wk{l}", _pack(qkv_w[l][:, D : 2 * D]))
        wts.add(f"wv{l}", _pack(qkv_w[l][:, 2 * D : 3 * D]))
        wts.add(f"wo{l}", _pack(out_w[l]))
        wts.add(f"wg{l}", _pack(gate_w[l]))
        for e in range(E):
            wts.add(f"w1_{l}_{e}", _pack(e_w1[l, e]))
            wts.add(f"w2_{l}_{e}", _pack(e_w2[l, e]))

    bia.add("ir_b1", _col(f("ir_b1")))
    bia.add("ir_b2", _col(f("ir_b2")))
    bia.add("p2m_b", _col(f("p2m_b")))
    for l in range(L):
        bia.add(f"ln1g{l}", _col(f("ln1_g")[l]))
        bia.add(f"ln1b{l}", _col(f("ln1_b")[l]))
        bia.add(f"ln2g{l}", _col(f("ln2_g")[l]))
        bia.add(f"ln2b{l}", _col(f("ln2_b")[l]))
        bia.add(f"qb{l}", _col(f("qkv_b")[l][0:D]))
        bia.add(f"kb{l}", _col(f("qkv_b")[l][D : 2 * D]))
        bia.add(f"ob{l}", _col(f("out_b")[l]))
        for e in range(E):
            bia.add(f"b1_{l}_{e}", _col(f("exp_b1")[l, e]))
    bia.add("fn_g", _col(f("fn_g")))
    bia.add("fn_b", _col(f("fn_b")))
    bia.add("head_g", _col(f("head_g")))
    bia.add("head_b", _col(f("head_b")))
    bia.add("head_w", _col(f("head_w")))
    bia.add("head_bias", np.full((1, 1), float(np.asarray(inp["head_bias"])), F32))
    bia.add("eps5", np.full((1, 1), 1e-5, F32))
    bia.add("eps6", np.full((1, 1), 1e-6, F32))

    for l in range(L):
        rows.add(f"vb{l}", f("qkv_b")[l][2 * D : 3 * D].reshape(1, D).astype(BF16))
        rows.add(f"gb{l}", f("gate_b")[l].reshape(1, E).astype(BF16))

    # exp_b2 combine lhsT stacks: [L, E, D] -> [E, L*D]
    b2s = np.ascontiguousarray(
        f("exp_b2").transpose(1, 0, 2).reshape(E, L * D)).astype(BF16)

    cons_f = _Packer(128, F32)
    cons_f.add("ones", np.ones((128, 256), F32))
    cons_f.add("ident", np.eye(128, dtype=F32))
    cons_b = _Packer(128, BF16)
    cons_b.add("ones", np.ones((128, 256), BF16))
    cons_b.add("mask", np.triu(np.ones((128, 128), F32)).astype(BF16))
    oh = np.zeros((E, E * 128), F32)
    for e in range(E):
        oh[e, e * 128 : (e + 1) * 128] = 1.0
    cons_b.add("oh", oh.astype(BF16))

    host = {
        "WTS": wts.finish(),
        "BIA": bia.finish(),
        "ROWS": rows.finish(),
        "B2S": b2s,
        "CONF": cons_f.finish(),
        "CONB": cons_b.finish(),
        "FEMB": f("freq_emb"),
    }
    offs = {"wts": wts.off, "bia": bia.off, "rows": rows.off,
            "conf": cons_f.off, "conb": cons_b.off}
    shapes = {k: v.shape for k, v in host.items()}
    return host, offs, shapes


def _per_core_inputs(inp, host):
    x = np.asarray(inp["x"], F32)
    fid = np.asarray(inp["freq_id"]).astype(np.int64)
    maps = []
    for c in range(NCORES):
        xc = x[c * BPC : (c + 1) * BPC]
        pt = np.ascontiguousarray(
            xc.reshape(BPC, N, P, C).transpose(2, 3, 0, 1).reshape(128, TOK))
        ohx = np.zeros((8, TOK), F32)
        for b in range(BPC):
            ohx[fid[c * BPC + b], b * N : (b + 1) * N] = 1.0
        m = dict(host)
        m["PT"] = pt
        m["OHX"] = ohx
        maps.append(m)
    return maps


# ------------------------------------------------------------- device build
def _build(offs, shapes, layers=L):
    import contextlib

    import concourse.mybir as mybir
    from concourse import bacc

    dt = mybir.dt
    AF = mybir.ActivationFunctionType
    OP = mybir.AluOpType
    AX = mybir.AxisListType
    AF_GELU = AF.Tanh if SIM_ACT_SWAP else AF.Gelu
    FixedTileContext = _fixed_tile_context()

    nc = bacc.Bacc("TRN2", target_bir_lowering=False, debug=False)
    T = {}
    T["WTS"] = nc.dram_tensor("WTS", list(shapes["WTS"]), dt.float16, kind="ExternalInput")
    T["BIA"] = nc.dram_tensor("BIA", list(shapes["BIA"]), dt.float32, kind="ExternalInput")
    T["ROWS"] = nc.dram_tensor("ROWS", list(shapes["ROWS"]), dt.float16, kind="ExternalInput")
    T["B2S"] = nc.dram_tensor("B2S", list(shapes["B2S"]), dt.float16, kind="ExternalInput")
    T["CONF"] = nc.dram_tensor("CONF", list(shapes["CONF"]), dt.float32, kind="ExternalInput")
    T["CONB"] = nc.dram_tensor("CONB", list(shapes["CONB"]), dt.float16, kind="ExternalInput")
    T["FEMB"] = nc.dram_tensor("FEMB", list(shapes["FEMB"]), dt.float32, kind="ExternalInput")
    T["PT"] = nc.dram_tensor("PT", [128, TOK], dt.float32, kind="ExternalInput")
    T["OHX"] = nc.dram_tensor("OHX", [8, TOK], dt.float32, kind="ExternalInput")
    T["LOGITS"] = nc.dram_tensor("LOGITS", [1, BPC], dt.float32, kind="ExternalOutput")
    T["PROBS"] = nc.dram_tensor("PROBS", [1, BPC], dt.float32, kind="ExternalOutput")

    WO, BO, RO = offs["wts"], offs["bia"], offs["rows"]
    CF, CB = offs["conf"], offs["conb"]

    with FixedTileContext(nc) as tc, contextlib.ExitStack() as ctx:
        sb = ctx.enter_context(tc.tile_pool(name="sb", bufs=1))
        ps = ctx.enter_context(tc.tile_pool(name="ps", bufs=1, space="PSUM"))
        # PSUM bank budget (8 banks, one slot = one bank):
        #   tag "mm"  bufs=3, tag "att" bufs=2, tag "moe" bufs=2, tag "row" bufs=1

        # ---------------- resident constants / biases
        ones_f = sb.tile([128, 256], dt.float32, tag="ones_f")
        nc.sync.dma_start(ones_f[:], T["CONF"][:, CF["ones"] : CF["ones"] + 256])
        ident = sb.tile([128, 128], dt.float32, tag="ident")
        nc.sync.dma_start(ident[:], T["CONF"][:, CF["ident"] : CF["ident"] + 128])
        ones_b = sb.tile([128, 256], dt.float16, tag="ones_b")
        nc.sync.dma_start(ones_b[:], T["CONB"][:, CB["ones"] : CB["ones"] + 256])
        mask_b = sb.tile([128, 128], dt.float16, tag="mask_b")
        nc.sync.dma_start(mask_b[:], T["CONB"][:, CB["mask"] : CB["mask"] + 128])
        oh_b = sb.tile([4, 512], dt.float16, tag="oh_b")
        nc.sync.dma_start(oh_b[:], T["CONB"][0:4, CB["oh"] : CB["oh"] + 512])
        bias_sb = sb.tile([128, shapes["BIA"][1]], dt.float32, tag="bias_sb")
        nc.sync.dma_start(bias_sb[:], T["BIA"][:])
        rows_sb = sb.tile([1, shapes["ROWS"][1]], dt.float16, tag="rows_sb")
        nc.sync.dma_start(rows_sb[:], T["ROWS"][0:1, :])
        femb_sb = sb.tile([8, 512], dt.float32, tag="femb_sb")
        nc.sync.dma_start(femb_sb[:], T["FEMB"][:])
        ohx_sb = sb.tile([8, TOK], dt.float32, tag="ohx_sb")
        nc.sync.dma_start(ohx_sb[:], T["OHX"][:])
        w_ir1 = sb.tile([128, 512], dt.float16, tag="w_ir1")
        nc.sync.dma_start(w_ir1[:], T["WTS"][:, WO["ir_w1"] : WO["ir_w1"] + 512])
        w_ir2 = sb.tile([128, 512], dt.float16, tag="w_ir2")
        nc.sync.dma_start(w_ir2[:], T["WTS"][:, WO["ir_w2"] : WO["ir_w2"] + 512])
        w_p2m = sb.tile([128, 512], dt.float16, tag="w_p2m")
        nc.sync.dma_start(w_p2m[:], T["WTS"][:, WO["p2m_w"] : WO["p2m_w"] + 512])

        def bcol(name, k=0):
            return bias_sb[:, BO[name] + k : BO[name] + k + 1]

        def rrow(name, w):
            return rows_sb[0:1, RO[name] : RO[name] + w]

        # ---------------- helpers
        def ln_rows(src_tiles, width, nfeat, eps_name, name=""):
            """Partition-dim mean/rstd across the given feature tiles for
            `width` token columns.  Returns psum [128, 2*width]: broadcast
            mean at [:, :width], broadcast rstd at [:, width:]."""
            nt = len(src_tiles)
            st = ps.tile([1, 2 * width], dt.float32, tag="att", bufs=3,
                         name=f"st{name}")
            ths = []
            for i, t in enumerate(src_tiles):
                th = sb.tile([128, width], dt.float16, tag="th", bufs=3,
                             name=f"th{name}{i}")
                nc.vector.tensor_copy(th[:, 0:width], t)
                ths.append(th)
            for i, th in enumerate(ths):
                nc.tensor.matmul(st[:, 0:width], ones_b[:, 0:1], th[:, 0:width],
                                 start=(i == 0), stop=(i == nt - 1))
            sqs = []
            for i, t in enumerate(src_tiles):
                sq = sb.tile([128, width], dt.float16, tag="sq", bufs=3,
                             name=f"sq{name}{i}")
                nc.scalar.activation(sq[:, 0:width], t, AF.Square)
                sqs.append(sq)
            for i, sq in enumerate(sqs):
                nc.tensor.matmul(st[:, width : 2 * width], ones_b[:, 0:1],
                                 sq[:, 0:width],
                                 start=(i == 0), stop=(i == nt - 1))
            r = sb.tile([1, 3 * width], dt.float32, tag="rows", bufs=2,
                        name=f"r{name}")
            r16 = sb.tile([1, 2 * width], dt.float16, tag="rows16", bufs=2,
                          name=f"r16{name}")
            mean = r16[:, 0:width]
            nc.vector.tensor_scalar_mul(mean, st[:, 0:width], 1.0 / nfeat)
            m2 = r[:, width : 2 * width]
            nc.vector.tensor_mul(m2, mean, mean)
            var = r[:, 2 * width : 3 * width]
            nc.vector.scalar_tensor_tensor(var, st[:, width : 2 * width],
                                           1.0 / nfeat, m2, OP.mult, OP.subtract)
            sd = r[:, width : 2 * width]      # reuse m2 slot
            nc.scalar.activation(sd, var, AF.Sqrt,
                                 bias=bias_sb[0:1, BO[eps_name] : BO[eps_name] + 1])
            rstd = r[:, 2 * width : 3 * width]  # reuse var slot
            nc.vector.reciprocal_approx_fast(out=rstd, in_=sd)
            rstd16 = r16[:, width : 2 * width]
            nc.vector.tensor_copy(rstd16, rstd)
            bc = ps.tile([128, 2 * width], dt.float32, tag="mm", bufs=4,
                         name=f"bc{name}")
            nc.tensor.matmul(bc[:, 0:width], ones_b[0:1, 0:128], mean,
                             start=True, stop=True)
            nc.tensor.matmul(bc[:, width : 2 * width], ones_b[0:1, 0:128], rstd16,
                             start=True, stop=True)
            return bc

        def layernorm(h_tiles, gname, bname, name=""):
            bc = ln_rows([t[:] for t in h_tiles], TOK, D, "eps5", name=name)
            outs = []
            for k, ht in enumerate(h_tiles):
                tmp = sb.tile([128, TOK], dt.float32, tag="lntmp", bufs=3,
                              name=f"lt{name}{k}")
                nc.vector.tensor_sub(tmp[:], ht[:], bc[:, 0:TOK])
                nc.vector.tensor_mul(tmp[:], tmp[:], bc[:, TOK : 2 * TOK])
                hn = sb.tile([128, TOK], dt.float16, tag="hn", bufs=12,
                             name=f"hn{name}{k}")
                nc.vector.tensor_scalar(hn[:], tmp[:], bcol(gname, k),
                                        bcol(bname, k), OP.mult, OP.add)
                outs.append(hn)
            return outs

        # ---------------- patch embedding
        pt = sb.tile([128, TOK], dt.float32, tag="pt")
        nc.sync.dma_start(pt[:], T["PT"][:])
        bc0 = ln_rows([pt[:]], TOK, PD, "eps6", name="pe")
        pn = sb.tile([128, TOK], dt.float32, tag="pn")
        nc.vector.tensor_sub(pn[:], pt[:], bc0[:, 0:TOK])
        nc.vector.tensor_mul(pn[:], pn[:], bc0[:, TOK : 2 * TOK])
        pn_bf = sb.tile([128, TOK], dt.float16, tag="pn_bf")
        nc.vector.tensor_copy(pn_bf[:], pn[:])

        gir = []
        for mt in range(4):
            p1 = ps.tile([128, TOK], dt.float32, tag="mm", bufs=4, name=f"pir{mt}")
            nc.tensor.matmul(p1[:, 0:TOK], w_ir1[:, mt * 128 : (mt + 1) * 128],
                             pn_bf[:], start=True, stop=True)
            g = sb.tile([128, TOK], dt.float16, tag="g", bufs=20, name=f"gir{mt}")
            nc.scalar.activation(g[:], p1[:, 0:TOK], AF_GELU, bias=bcol("ir_b1", mt))
            gir.append(g)
        p2 = ps.tile([128, TOK], dt.float32, tag="mm", bufs=4, name="pir2")
        for k in range(4):
            nc.tensor.matmul(p2[:, 0:TOK], w_ir2[:, k * 128 : (k + 1) * 128],
                             gir[k][:], start=(k == 0), stop=(k == 3))
        hp = sb.tile([128, TOK], dt.float32, tag="hp")
        nc.vector.scalar_tensor_tensor(hp[:], p2[:, 0:TOK], bcol("ir_b2", 0),
                                       pn[:], OP.add, OP.add)
        hp_bf = sb.tile([128, TOK], dt.float16, tag="hp_bf")
        nc.vector.tensor_copy(hp_bf[:], hp[:])

        h_tiles = []
        for mt in range(4):
            p3 = ps.tile([128, TOK], dt.float32, tag="mm", bufs=4, name=f"pm{mt}")
            nc.tensor.matmul(p3[:, 0:TOK], w_p2m[:, mt * 128 : (mt + 1) * 128],
                             hp_bf[:], start=True, stop=False)
            nc.tensor.matmul(p3[:, 0:TOK], femb_sb[:, mt * 128 : (mt + 1) * 128],
                             ohx_sb[:], start=False, stop=True)
            ht = sb.tile([128, TOK], dt.float32, tag="h", bufs=8, name=f"h0_{mt}")
            nc.vector.tensor_scalar_add(ht[:], p3[:, 0:TOK], bcol("p2m_b", mt))
            h_tiles.append(ht)

        # ---------------- transformer layers
        for l in range(layers):
            wq = sb.tile([128, 2048], dt.float16, tag="wq", bufs=3, name=f"wq{l}")
            nc.sync.dma_start(wq[:], T["WTS"][:, WO[f"wq{l}"] : WO[f"wq{l}"] + 2048])
            wk = sb.tile([128, 2048], dt.float16, tag="wk", bufs=3, name=f"wk{l}")
            nc.sync.dma_start(wk[:], T["WTS"][:, WO[f"wk{l}"] : WO[f"wk{l}"] + 2048])
            wv = sb.tile([128, 2048], dt.float16, tag="wv", bufs=3, name=f"wv{l}")
            nc.sync.dma_start(wv[:], T["WTS"][:, WO[f"wv{l}"] : WO[f"wv{l}"] + 2048])
            wo = sb.tile([128, 2048], dt.float16, tag="wo", bufs=3, name=f"wo{l}")
            nc.sync.dma_start(wo[:], T["WTS"][:, WO[f"wo{l}"] : WO[f"wo{l}"] + 2048])
            wg = sb.tile([128, 16], dt.float16, tag="wg", bufs=3, name=f"wg{l}")
            nc.sync.dma_start(wg[:], T["WTS"][:, WO[f"wg{l}"] : WO[f"wg{l}"] + 16])
            b2 = sb.tile([4, 512], dt.float16, tag="b2", bufs=3, name=f"b2_{l}")
            nc.sync.dma_start(b2[:], T["B2S"][0:4, l * 512 : (l + 1) * 512])

            # -- attention
            hn1 = layernorm(h_tiles, f"ln1g{l}", f"ln1b{l}", name=f"a{l}")

            qt, kt_ = [], []
            for which, wmat, bn, dst in (("q", wq, f"qb{l}", qt),
                                         ("k", wk, f"kb{l}", kt_)):
                for mt in range(4):
                    pq = ps.tile([128, TOK], dt.float32, tag="mm", bufs=4,
                                 name=f"p{which}{l}_{mt}")
                    for k in range(4):
                        nc.tensor.matmul(
                            pq[:, 0:TOK],
                            wmat[:, k * 512 + mt * 128 : k * 512 + (mt + 1) * 128],
                            hn1[k][:], start=(k == 0), stop=(k == 3))
                    q_sb = sb.tile([128, TOK], dt.float16, tag="qk", bufs=10,
                                   name=f"{which}{l}_{mt}")
                    nc.vector.tensor_scalar_add(q_sb[:], pq[:, 0:TOK], bcol(bn, mt))
                    dst.append(q_sb)
            vt = []
            for b in range(BPC):
                pv = ps.tile([128, 512], dt.float32, tag="mm", bufs=4,
                             name=f"pv{l}_{b}")
                for k in range(4):
                    nc.tensor.matmul(pv[:], hn1[k][:, b * N : (b + 1) * N],
                                     wv[:, k * 512 : (k + 1) * 512],
                                     start=(k == 0), stop=False)
                nc.tensor.matmul(pv[:], ones_b[0:1, 0:128], rrow(f"vb{l}", D),
                                 start=False, stop=True)
                v_sb = sb.tile([128, 512], dt.float16, tag="v", bufs=3,
                               name=f"v{l}_{b}")
                nc.vector.tensor_copy(v_sb[:], pv[:])
                vt.append(v_sb)

            o_tiles = [sb.tile([128, TOK], dt.float16, tag="o", bufs=6,
                               name=f"o{l}_{j}") for j in range(4)]
            for b in range(BPC):
                bs = slice(b * N, (b + 1) * N)
                for j in range(4):  # head pair (2j, 2j+1) = D-tile j
                    pr0 = ps.tile([128, N], dt.float32, tag="att", bufs=3,
                                  name=f"s{l}_{b}_{j}0")
                    nc.tensor.matmul(pr0[:], kt_[j][0:64, bs], qt[j][0:64, bs],
                                     start=True, stop=True)
                    pr1 = ps.tile([128, N], dt.float32, tag="att", bufs=3,
                                  name=f"s{l}_{b}_{j}1")
                    nc.tensor.matmul(pr1[:], kt_[j][64:128, bs], qt[j][64:128, bs],
                                     start=True, stop=True, tile_position=(64, 0))
                    a0 = sb.tile([128, N], dt.float16, tag="a", bufs=8,
                                 name=f"a{l}_{b}_{j}0")
                    nc.scalar.activation(a0[:], pr0[:], AF.Exp, scale=0.125)
                    nc.vector.tensor_mul(a0[:], a0[:], mask_b[:])
                    a1 = sb.tile([128, N], dt.float16, tag="a", bufs=8,
                                 name=f"a{l}_{b}_{j}1")
                    nc.scalar.activation(a1[:], pr1[:], AF.Exp, scale=0.125)
                    nc.vector.tensor_mul(a1[:], a1[:], mask_b[:])
                    pd_ = ps.tile([128, N], dt.float32, tag="att", bufs=3,
                                  name=f"d{l}_{b}_{j}")
                    nc.tensor.matmul(pd_[0:64, :], ones_b[:, 0:64], a0[:],
                                     start=True, stop=True)
                    nc.tensor.matmul(pd_[64:128, :], ones_b[:, 64:128], a1[:],
                                     start=True, stop=True, tile_position=(0, 64))
                    rec = sb.tile([128, N], dt.float32, tag="rec", bufs=4,
                                  name=f"rc{l}_{b}_{j}")
                    nc.vector.reciprocal_approx_fast(out=rec[:], in_=pd_[:])
                    po = ps.tile([128, N], dt.float32, tag="att", bufs=3,
                                 name=f"po{l}_{b}_{j}")
                    nc.tensor.matmul(po[0:64, :],
                                     vt[b][:, 128 * j : 128 * j + 64],
                                     a0[:], start=True, stop=True)
                    nc.tensor.matmul(po[64:128, :],
                                     vt[b][:, 128 * j + 64 : 128 * j + 128],
                                     a1[:], start=True, stop=True,
                                     tile_position=(0, 64))
                    nc.vector.tensor_mul(o_tiles[j][:, bs], po[:], rec[:])

            for mt in range(4):
                pu = ps.tile([128, TOK], dt.float32, tag="mm", bufs=4,
                             name=f"pu{l}_{mt}")
                for k in range(4):
                    nc.tensor.matmul(
                        pu[:, 0:TOK],
                        wo[:, k * 512 + mt * 128 : k * 512 + (mt + 1) * 128],
                        o_tiles[k][:], start=(k == 0), stop=(k == 3))
                hnew = sb.tile([128, TOK], dt.float32, tag="h", bufs=8,
                               name=f"ha{l}_{mt}")
                nc.vector.scalar_tensor_tensor(hnew[:], pu[:, 0:TOK],
                                               bcol(f"ob{l}", mt), h_tiles[mt][:],
                                               OP.add, OP.add)
                h_tiles[mt] = hnew

            # -- MoE
            hn2 = layernorm(h_tiles, f"ln2g{l}", f"ln2b{l}", name=f"m{l}")

            wgt_tm = []
            for tb in range(BPC):
                pg = ps.tile([128, E], dt.float32, tag="att", bufs=3,
                             name=f"pg{l}_{tb}")
                for k in range(4):
                    nc.tensor.matmul(pg[:], hn2[k][:, tb * N : (tb + 1) * N],
                                     wg[:, k * E : (k + 1) * E],
                                     start=(k == 0), stop=False)
                nc.tensor.matmul(pg[:], ones_b[0:1, 0:128], rrow(f"gb{l}", E),
                                 start=False, stop=True)
                w_ = sb.tile([128, 12], dt.float32, tag="gate", bufs=4,
                             name=f"gw{l}_{tb}")
                nc.scalar.activation(w_[:, 0:4], pg[:], AF.Exp)
                nc.vector.tensor_reduce(w_[:, 4:5], w_[:, 0:4], axis=AX.X, op=OP.add)
                nc.vector.reciprocal_approx_fast(out=w_[:, 5:6], in_=w_[:, 4:5])
                nc.vector.tensor_scalar_mul(w_[:, 0:4], w_[:, 0:4], w_[:, 5:6])
                nc.vector.tensor_reduce(w_[:, 4:5], w_[:, 0:4], axis=AX.X, op=OP.max)
                nc.vector.tensor_scalar(w_[:, 6:10], w_[:, 0:4], w_[:, 4:5],
                                        -1e30, OP.is_ge, OP.mult)
                nc.vector.tensor_add(w_[:, 6:10], w_[:, 6:10], w_[:, 0:4])
                nc.vector.tensor_reduce(w_[:, 10:11], w_[:, 6:10], axis=AX.X,
                                        op=OP.max)
                wgt = sb.tile([128, E], dt.float32, tag="wgt", bufs=4,
                              name=f"wgt{l}_{tb}")
                nc.vector.scalar_tensor_tensor(wgt[:], w_[:, 0:4], w_[:, 10:11],
                                               w_[:, 0:4], OP.is_ge, OP.mult)
                wgt_tm.append(wgt)
            pwt = ps.tile([4, TOK], dt.float32, tag="att", bufs=3, name=f"pwt{l}")
            for tb in range(BPC):
                nc.tensor.transpose(pwt[0:4, tb * N : (tb + 1) * N],
                                    wgt_tm[tb][:, 0:4], ident[:])
            wgt_t = sb.tile([4, TOK], dt.float16, tag="wgt_t", bufs=2,
                            name=f"wgtt{l}")
            nc.vector.tensor_copy(wgt_t[:], pwt[0:4, :])
            wbs = []
            for e in range(E):
                pwb = ps.tile([128, TOK], dt.float32, tag="mm", bufs=4,
                              name=f"pwb{l}_{e}")
                nc.tensor.matmul(pwb[:, 0:TOK], oh_b[:, e * 128 : (e + 1) * 128],
                                 wgt_t[:], start=True, stop=True)
                wb = sb.tile([128, TOK], dt.float16, tag="wb", bufs=6,
                             name=f"wb{l}_{e}")
                nc.vector.tensor_copy(wb[:], pwb[:, 0:TOK])
                wbs.append(wb)

            for e in range(E):
                w1 = sb.tile([128, 8192], dt.float16, tag="w1", bufs=2,
                             name=f"w1_{l}_{e}")
                nc.sync.dma_start(
                    w1[:], T["WTS"][:, WO[f"w1_{l}_{e}"] : WO[f"w1_{l}_{e}"] + 8192])
                w2 = sb.tile([128, 8192], dt.float16, tag="w2", bufs=2,
                             name=f"w2_{l}_{e}")
                nc.sync.dma_start(
                    w2[:], T["WTS"][:, WO[f"w2_{l}_{e}"] : WO[f"w2_{l}_{e}"] + 8192])
                gts = []
                for mt in range(HT):
                    ph = ps.tile([128, TOK], dt.float32, tag="mm", bufs=4,
                                 name=f"ph{l}_{e}_{mt}")
                    for k in range(4):
                        nc.tensor.matmul(
                            ph[:, 0:TOK],
                            w1[:, k * 2048 + mt * 128 : k * 2048 + (mt + 1) * 128],
                            hn2[k][:], start=(k == 0), stop=(k == 3))
                    g = sb.tile([128, TOK], dt.float16, tag="g", bufs=20,
                                name=f"g{l}_{e}_{mt}")
                    nc.scalar.activation(g[:], ph[:, 0:TOK], AF_GELU,
                                         bias=bcol(f"b1_{l}_{e}", mt))
                    nc.vector.tensor_mul(g[:], g[:], wbs[e][:])
                    gts.append(g)
                for mt in range(4):
                    pm = ps.tile([128, TOK], dt.float32, tag="mm", bufs=4,
                                 name=f"pmoe{l}_{e}_{mt}")
                    if e == 0:
                        nc.tensor.matmul(pm[:, 0:TOK],
                                         b2[:, mt * 128 : (mt + 1) * 128],
                                         wgt_t[:], start=True, stop=False)
                    for k in range(HT):
                        nc.tensor.matmul(
                            pm[:, 0:TOK],
                            w2[:, k * 512 + mt * 128 : k * 512 + (mt + 1) * 128],
                            gts[k][:],
                            start=(e != 0 and k == 0),
                            stop=(k == HT - 1))
                    hnew = sb.tile([128, TOK], dt.float32, tag="h", bufs=8,
                                   name=f"hm{l}_{e}_{mt}")
                    nc.vector.tensor_add(hnew[:], pm[:, 0:TOK], h_tiles[mt][:])
                    h_tiles[mt] = hnew

        # ---------------- head (last token of each sequence)
        cur = [h_tiles[k][:, N - 1 :: N] for k in range(4)]  # [128, BPC] views
        for pass_i, (gn, bn) in enumerate((("fn_g", "fn_b"),
                                           ("head_g", "head_b"))):
            bc = ln_rows(cur, BPC, D, "eps5", name=f"hd{pass_i}")
            new_tiles = []
            for k in range(4):
                t2 = sb.tile([128, BPC], dt.float32, tag="hl", bufs=8,
                             name=f"hl{pass_i}_{k}")
                nc.vector.tensor_sub(t2[:], cur[k], bc[:, 0:BPC])
                nc.vector.tensor_mul(t2[:], t2[:], bc[:, BPC : 2 * BPC])
                nc.vector.tensor_scalar(t2[:], t2[:], bcol(gn, k), bcol(bn, k),
                                        OP.mult, OP.add)
                new_tiles.append(t2[:])
            cur = new_tiles

        plg = ps.tile([1, BPC], dt.float32, tag="att", bufs=3, name="plg")
        for k in range(4):
            nc.tensor.matmul(plg[:], bcol("head_w", k), cur[k],
                             start=(k == 0), stop=(k == 3))
        lg = sb.tile([1, BPC], dt.float32, tag="lg")
        nc.vector.tensor_scalar_add(lg[:], plg[:],
                                    bias_sb[0:1, BO["head_bias"] : BO["head_bias"] + 1])
        pr = sb.tile([1, BPC], dt.float32, tag="pr")
        nc.scalar.activation(pr[:], lg[:], AF.Sigmoid)
        nc.sync.dma_start(T["LOGITS"][:], lg[:])
        nc.sync.dma_start(T["PROBS"][:], pr[:])

    nc.finalize()
    return nc, T


# ----------------------------------------------------------------- driver
def _get_program(inputs, layers=L):
    key = ("prog", layers, SIM_ACT_SWAP)
    if key not in _CACHE:
        host, offs, shapes = _prep_host(inputs)
        nc, T = _build(offs, shapes, layers=layers)
        _CACHE[key] = (nc, offs, shapes)
        _CACHE[("host", layers)] = host
    return _CACHE[key], _CACHE[("host", layers)]


def run_layers(inputs, layers=L, **run_kw):
    from concourse.bass_utils import run_bass_kernel_spmd

    (nc, offs, shapes), host = _get_program(inputs, layers=layers)
    in_maps = _per_core_inputs(inputs, host)
    res = run_bass_kernel_spmd(nc, in_maps, core_ids=list(range(NCORES)), **run_kw)
    logits = np.concatenate([r["LOGITS"].reshape(-1) for r in res.results])
    probs = np.concatenate([r["PROBS"].reshape(-1) for r in res.results])
    return (logits.astype(F32), probs.astype(F32)), res


def kernel(**inputs):
    out, _ = run_layers(inputs, L)
    return out

